# revision 1
# baseline (speedup 1.0000x reference)
"""ConvLSTM2D + residual block on 8 TRN2 NeuronCores.

Strategy: data-parallel over batch B=8 (one batch element per core, no
collectives).  Per core everything is channel-major [C, H*W]:

- AH buffer [128, 66*66] fp32r: rows 0:64 = zero-padded x_t, rows 64:128 =
  zero-padded h_{t-1}.  The two convolutions (x*Wx + h*Wh) become 9
  accumulating matmuls per output tile with K=128 (64 x-channels + 64
  h-channels stacked), reading shifted windows of AH -- a 3x3 SAME conv is
  just 9 offsets into the padded buffer.
- Gate order is packed as PSUM bank A = [z_i; z_f], bank B = [z_g; z_o] so
  hard-sigmoid/tanh/elementwise LSTM math runs on [128, n] tiles.
- fp32r matmuls: 4x faster than fp32 on the PE (1 cycle/row), ~1.5e-4 rel
  error.  All fp32r data is produced by rounding compute ops (DVE/ACT),
  never raw DMA.
"""
import sys
sys.path.insert(0, '/opt/trn_rl_repo')
import numpy as np

B, T, H, W, C = 8, 14, 64, 64, 64
HW = H * W            # 4096
PW = 66               # padded H/W
NPOS = PW * PW        # 4356
NCHUNK = 4            # spatial chunks per step (16 image rows each)
CH = HW // NCHUNK     # 1024 positions per chunk

_CACHE = {}


def _build():
    from concourse import bacc, mybir, tile
    from concourse.alu_op_type import AluOpType as ALU
    F32 = mybir.dt.float32
    F32R = mybir.dt.float32r
    AF = mybir.ActivationFunctionType

    nc = bacc.Bacc("TRN2", target_bir_lowering=False)
    XD = nc.dram_tensor("xcm", [T, 64, HW], F32, kind="ExternalInput")
    WD = nc.dram_tensor("wc", [128, 9 * 256], F32, kind="ExternalInput")
    BD = nc.dram_tensor("bias", [128, 2], F32, kind="ExternalInput")
    YD = nc.dram_tensor("ycm", [T, 64, HW], F32, kind="ExternalOutput")

    with tile.TileContext(nc) as tc:
        with tc.tile_pool(name="const", bufs=1) as cp, \
             tc.tile_pool(name="work", bufs=2) as wk, \
             tc.tile_pool(name="xst", bufs=2) as xp, \
             tc.tile_pool(name="yst", bufs=2) as yp, \
             tc.tile_pool(name="ps", bufs=2, space="PSUM") as ps:

            wst = cp.tile([128, 9 * 256], F32)
            nc.sync.dma_start(wst[:], WD[:, :])
            wt = cp.tile([128, 9 * 256], F32R)
            nc.vector.tensor_copy(wt[:], wst[:])
            bt = cp.tile([128, 2], F32)
            nc.sync.dma_start(bt[:], BD[:, :])

            ahA = cp.tile([128, NPOS], F32R)
            ahB = cp.tile([128, NPOS], F32R)
            nc.vector.memset(ahA[:].bitcast(F32), 0.0)
            nc.vector.memset(ahB[:].bitcast(F32), 0.0)
            GC = cp.tile([128, HW], F32)   # rows 0:64 = tanh(z_g), rows 64:128 = c state
            nc.vector.memset(GC[:], 0.0)

            for t in range(T):
                ah_cur = ahA if t % 2 == 0 else ahB
                ah_nxt = ahB if t % 2 == 0 else ahA
                ahv_cur = ah_cur[:].rearrange("p (r c) -> p r c", c=PW)
                ahv_nxt = ah_nxt[:].rearrange("p (r c) -> p r c", c=PW)

                # stage x_t and round it into the padded x-half of AH
                xs = xp.tile([64, HW], F32)
                nc.sync.dma_start(xs[:], XD[t, :, :])
                nc.vector.tensor_copy(
                    ahv_cur[0:64, 1:65, 1:65],
                    xs[:].rearrange("p (r c) -> p r c", c=W))

                YT = yp.tile([64, HW], F32)

                for q in range(NCHUNK):
                    cols = slice(q * CH, (q + 1) * CH)
                    zA = ps.tile([128, CH], F32, tag="zA")
                    zB = ps.tile([128, CH], F32, tag="zB")
                    for mt, z in ((0, zA), (1, zB)):
                        for nh in range(2):
                            r0 = q * 16 + nh * 8
                            for tap in range(9):
                                dy, dx = tap // 3, tap % 3
                                nc.tensor.matmul(
                                    z[:, nh * 512:(nh + 1) * 512],
                                    wt[:, tap * 256 + mt * 128:
                                       tap * 256 + (mt + 1) * 128],
                                    ahv_cur[:, r0 + dy:r0 + 8 + dy, dx:dx + 64],
                                    start=(tap == 0), stop=(tap == 8))

                    # gates: i,f = hsig(zA + b_if); g = tanh(zB_lo + b_g);
                    # o = hsig(zB_hi + b_o); c' = f*c + i*g; h = o*tanh(c')
                    IF = wk.tile([128, CH], F32)
                    nc.vector.tensor_scalar(IF[:], zA[:], 0.2, bt[:, 0:1],
                                            ALU.mult, ALU.add)
                    nc.vector.tensor_scalar(IF[:], IF[:], 0.0, 1.0,
                                            ALU.max, ALU.min)
                    nc.scalar.activation(GC[0:64, cols], zB[0:64, :], AF.Tanh,
                                         bias=bt[0:64, 1:2])
                    OT = wk.tile([64, CH], F32)
                    nc.scalar.activation(OT[:], zB[64:128, :], AF.Relu,
                                         bias=bt[64:128, 1:2], scale=0.2)
                    nc.vector.tensor_scalar_min(OT[:], OT[:], 1.0)

                    U = wk.tile([128, CH], F32)
                    nc.vector.tensor_tensor(U[:], IF[:], GC[:, cols], ALU.mult)
                    V = wk.tile([128, CH], F32)
                    nc.scalar.activation(V[64:128, :], U[0:64, :], AF.Copy)
                    nc.vector.tensor_tensor(GC[64:128, cols], V[64:128, :],
                                            U[64:128, :], ALU.add)
                    TC = wk.tile([64, CH], F32)
                    nc.scalar.activation(TC[:], GC[64:128, cols], AF.Tanh)
                    HT = wk.tile([64, CH], F32)
                    nc.vector.tensor_tensor(HT[:], OT[:], TC[:], ALU.mult)
                    # h -> padded h-half of the *next* AH buffer (rounds to f32r)
                    r0 = q * 16
                    nc.scalar.activation(
                        ahv_nxt[64:128, 1 + r0:1 + r0 + 16, 1:65],
                        HT[:].rearrange("p (r c) -> p r c", c=W), AF.Copy)
                    # residual
                    nc.vector.tensor_tensor(YT[:, cols], HT[:], xs[:, cols],
                                            ALU.add)

                nc.sync.dma_start(YD[t, :, :], YT[:])

    nc.compile()
    return nc


def _prep_inputs(x, Wx, Wh, b):
    # wc[k, tap*256 + m]: rows 0:64 = Wx[dy,dx], rows 64:128 = Wh[dy,dx]
    wx = np.ascontiguousarray(Wx, np.float32).reshape(9, 64, 256)
    wh = np.ascontiguousarray(Wh, np.float32).reshape(9, 64, 256)
    wc = np.concatenate([wx, wh], axis=1)          # [9, 128, 256]
    wc = np.ascontiguousarray(wc.transpose(1, 0, 2)).reshape(128, 9 * 256)

    b = np.asarray(b, np.float32)
    bias = np.empty((128, 2), np.float32)
    bias[:, 0] = 0.2 * b[0:128] + 0.5                      # i, f
    bias[0:64, 1] = b[128:192]                             # g
    bias[64:128, 1] = 0.2 * b[192:256] + 0.5               # o
    return wc, bias


def kernel(x, Wx, Wh, b):
    from concourse.bass_utils import run_bass_kernel_spmd

    if "nc" not in _CACHE:
        _CACHE["nc"] = _build()
    nc = _CACHE["nc"]

    x = np.asarray(x, np.float32)
    wc, bias = _prep_inputs(x, Wx, Wh, b)

    in_maps = []
    for bi in range(B):
        xcm = np.ascontiguousarray(
            x[bi].transpose(0, 3, 1, 2)).reshape(T, 64, HW)
        in_maps.append({"xcm": xcm, "wc": wc, "bias": bias})

    res = run_bass_kernel_spmd(nc, in_maps, core_ids=list(range(B)))
    _CACHE["last_results"] = res

    y = np.empty((B, T, H, W, C), np.float32)
    for bi in range(B):
        y[bi] = res.results[bi]["ycm"].reshape(T, C, H, W).transpose(0, 2, 3, 1)
    return y
